# revision 15
# baseline (speedup 1.0000x reference)
"""Gated multi-head self-attention on 8 Trainium2 NeuronCores.

Reference computation (per batch b of 4, N=1024 tokens, 8 heads x 64):
    q  = (x @ wq.T) * 64**-0.5            # scale folded into wqT on host
    k,v = split(x @ wkv.T)
    dots = q k^T + bias;  attn = softmax(dots)
    out  = (attn @ v) * sigmoid(x @ wg.T + bg)
    y    = out @ wo.T + bo                # bo added on host after gather

Sharding: token-sharded, zero collectives. Core c handles batch b=c//2 and
query-token half c%2 (512 query rows). Each core computes K/V for its whole
batch (the KV projection is duplicated across the 2 cores of a batch; this
is far cheaper than any collective at these sizes).

Layout trick: everything on-device lives channel-major ("transposed") --
qT/kT from wT.T @ xT, the attention output directly in [channel, token]
form -- so no on-device transposes are needed anywhere.  Softmax is done
unnormalized (exp with no max-subtraction: logits are O(5), safe in fp32)
as exp(dots) * exp(bias) where exp(bias^T) is precomputed on host in fp16.
Denominators come for free from 64 ones-columns appended to V inside the
AV matmul (PSUM rows 64:128 = 64 copies of the softmax denominator).
All matmuls run in float32r (full-rate at free dim >= 256).
"""

import sys

if "/opt/trn_rl_repo" not in sys.path:
    sys.path.insert(0, "/opt/trn_rl_repo")

import numpy as np

import concourse.bass as bass  # noqa: F401  (AP helpers)
import concourse.mybir as mybir
import concourse.tile as tile
from concourse import bacc
from concourse.bass_utils import run_bass_kernel_spmd

F32 = mybir.dt.float32
F32R = mybir.dt.float32r
F16 = mybir.dt.float16
AF = mybir.ActivationFunctionType
ALU = mybir.AluOpType

P = 128
HEADS = 8
DH = 64
DIM = 512
N = 1024  # tokens per batch (kv length)
NQ = 512  # query tokens per core
B = 4
N_CORES = 8


def build_nc(use_tile_position=True):
    nc = bacc.Bacc(None, target_bir_lowering=False, debug=False)

    # Per-core inputs. Token order inside xbT/ebT is "query half first".
    xbT_d = nc.dram_tensor("xbT", [DIM, N], F32R, kind="ExternalInput")
    bT_d = nc.dram_tensor("bT", [HEADS, N, NQ], F16, kind="ExternalInput")
    ident_d = nc.dram_tensor("ident", [P, P], F16, kind="ExternalInput")
    wqT_d = nc.dram_tensor("wqT", [DIM, DIM], F32R, kind="ExternalInput")
    wkT_d = nc.dram_tensor("wkT", [DIM, DIM], F32R, kind="ExternalInput")
    wvT_d = nc.dram_tensor("wvT", [DIM, DIM], F32R, kind="ExternalInput")
    wgT_d = nc.dram_tensor("wgT", [DIM, DIM], F32R, kind="ExternalInput")
    woT_d = nc.dram_tensor("woT", [DIM, DIM], F32R, kind="ExternalInput")
    bg_d = nc.dram_tensor("bg", [DIM], F32, kind="ExternalInput")
    ones_d = nc.dram_tensor("ones_v", [P, DH], F16, kind="ExternalInput")
    y_d = nc.dram_tensor("y", [NQ, DIM], F32, kind="ExternalOutput")

    DT = DIM // P  # 4 channel tiles of 128
    JT = N // P  # 8 kv-token tiles of 128

    with tile.TileContext(nc) as tc:
        with (
            tc.tile_pool(name="const", bufs=1) as const,
            tc.tile_pool(name="work", bufs=1) as work,
            tc.tile_pool(name="attn", bufs=16) as attn_pool,
            tc.tile_pool(name="rec", bufs=4) as rec_pool,
            tc.tile_pool(name="ebuf", bufs=6) as ebuf,
            tc.tile_pool(name="yout", bufs=2) as yout,
        ):
            # ---- constants ----------------------------------------------
            xbT = [const.tile([P, N], F32R, tag=f"xbT{t}", name=f"xbT{t}") for t in range(DT)]
            for t in range(DT):
                nc.sync.dma_start(xbT[t][:], xbT_d[t * P : (t + 1) * P, :])

            def load_w(name, dram, eng):
                ts_ = [const.tile([P, DIM], F32R, tag=f"{name}{t}", name=f"{name}{t}") for t in range(DT)]
                for t in range(DT):
                    eng.dma_start(ts_[t][:], dram[t * P : (t + 1) * P, :])
                return ts_

            wkT = load_w("wkT", wkT_d, nc.scalar)
            wvT = load_w("wvT", wvT_d, nc.gpsimd)
            wqT = load_w("wqT", wqT_d, nc.scalar)
            wgT = load_w("wgT", wgT_d, nc.gpsimd)
            woT = load_w("woT", woT_d, nc.scalar)

            ident_sb = const.tile([P, P], F16, tag="ident", name="ident")
            nc.sync.dma_start(ident_sb[:], ident_d[:, :])

            bg_sb = const.tile([P, DT], F32, tag="bg", name="bg")
            nc.sync.dma_start(bg_sb[:], bg_d.rearrange("(o p) -> p o", p=P))

            # persistent activations
            kT = [work.tile([P, N], F32R, tag=f"kT{t}", name=f"kT{t}") for t in range(DT)]
            v_aug = [work.tile([P, HEADS * P], F16, tag=f"vaug{j}", name=f"vaug{j}") for j in range(JT)]
            qT = [work.tile([P, NQ], F32R, tag=f"qT{t}", name=f"qT{t}") for t in range(DT)]
            sigT = [work.tile([P, NQ], F32, tag=f"sig{t}", name=f"sig{t}") for t in range(DT)]
            gatedT = [work.tile([P, NQ], F32R, tag=f"gated{t}", name=f"gated{t}") for t in range(DT)]

            # ---- PE warmup: get HAM to K=8/8 during the initial DMAs ----
            warm_sb = const.tile([P, 512], F16, tag="warm", name="warm")
            nc.sync.dma_start(
                warm_sb[:], ones_d[:, None, :].to_broadcast((P, HEADS, DH))
            )

            # ---- projections --------------------------------------------
            with (
                tc.tile_pool(name="ps_warm", bufs=1, space="PSUM") as ps_warm,
                tc.tile_pool(name="ps_proj", bufs=4, space="PSUM") as ps_proj,
            ):
                warm_ps = ps_warm.tile([P, 512], F32, tag="warm", name="warm_ps")
                for _ in range(16):
                    nc.tensor.matmul(
                        warm_ps[:], warm_sb[:, 0:P], warm_sb[:], start=True, stop=True
                    )
                # consumer so the warmup chain is not dead code
                nc.any.tensor_copy(out=warm_sb[0:1, 0:1], in_=warm_ps[0:1, 0:1])

                # kT[ct][:, j] = sum_d wkT[d, ct].T @ xbT[d, j]
                for ct in range(DT):
                    for jc in range(2):
                        ps = ps_proj.tile([P, 512], F32, tag="proj", name="proj")
                        for kt in range(DT):
                            nc.tensor.matmul(
                                ps[:],
                                wkT[kt][:, ct * P : (ct + 1) * P],
                                xbT[kt][:, jc * 512 : (jc + 1) * 512],
                                start=(kt == 0),
                                stop=(kt == DT - 1),
                            )
                        nc.any.tensor_copy(
                            out=kT[ct][:, jc * 512 : (jc + 1) * 512], in_=ps[:]
                        )

                # v_aug[j]: per head h cols h*128..h*128+64 = V, +64..+128 = ones
                for jt in range(JT):
                    nc.sync.dma_start(
                        v_aug[jt].rearrange("p (h c) -> p h c", c=P)[:, :, DH:P],
                        ones_d[:, None, :].to_broadcast((P, HEADS, DH)),
                    )
                for jt in range(JT):
                    ps = ps_proj.tile([P, 512], F32, tag="proj", name="proj")
                    for kt in range(DT):
                        nc.tensor.matmul(
                            ps[:],
                            xbT[kt][:, jt * P : (jt + 1) * P],
                            wvT[kt][:],
                            start=(kt == 0),
                            stop=(kt == DT - 1),
                        )
                    nc.any.tensor_copy(
                        out=v_aug[jt].rearrange("p (h c) -> p h c", c=P)[:, :, 0:DH],
                        in_=ps[:].rearrange("p (h c) -> p h c", c=DH),
                    )

                # qT (wq already scaled by 1/8 on host)
                for ct in range(DT):
                    ps = ps_proj.tile([P, 512], F32, tag="proj", name="proj")
                    for kt in range(DT):
                        nc.tensor.matmul(
                            ps[:],
                            wqT[kt][:, ct * P : (ct + 1) * P],
                            xbT[kt][:, 0:NQ],
                            start=(kt == 0),
                            stop=(kt == DT - 1),
                        )
                    nc.any.tensor_copy(out=qT[ct][:], in_=ps[:])

                # gates -> sigmoid(g + bg) directly from PSUM
                for ct in range(DT):
                    ps = ps_proj.tile([P, 512], F32, tag="proj", name="proj")
                    for kt in range(DT):
                        nc.tensor.matmul(
                            ps[:],
                            wgT[kt][:, ct * P : (ct + 1) * P],
                            xbT[kt][:, 0:NQ],
                            start=(kt == 0),
                            stop=(kt == DT - 1),
                        )
                    nc.scalar.activation(
                        out=sigT[ct][:],
                        in_=ps[:],
                        func=AF.Sigmoid,
                        bias=bg_sb[:, ct : ct + 1],
                    )

            # ---- attention (head pairs) ---------------------------------
            with (
                tc.tile_pool(name="ps_dots", bufs=2, space="PSUM") as ps_dots,
                tc.tile_pool(name="ps_av", bufs=4, space="PSUM") as ps_av,
            ):
                def emit_dots_pair(hp):
                    """bias-inject + QK^T + exp for heads 2hp, 2hp+1.

                    Returns 8 fp16 attn tiles [P(j), 2*NQ] (even head cols
                    0:NQ, odd head NQ:2NQ), unnormalized exp(dots+bias)."""
                    ct = hp  # channel tile holding this head pair
                    tiles = []
                    for jt in range(JT):
                        eb = ebuf.tile([P, 2, NQ], F16, tag="eb", name="eb")
                        nc.sync.dma_start(
                            eb[:],
                            bT_d[2 * hp : 2 * hp + 2, jt * P : (jt + 1) * P, :]
                            .rearrange("h p i -> p h i"),
                        )
                        dps = ps_dots.tile([P, 2 * NQ], F32, tag="dots", name="dots")
                        for s in range(2):
                            lo = s * DH
                            reg = dps[:, s * NQ : (s + 1) * NQ]
                            # psum = bias (identity matmul), then += q.k
                            nc.tensor.matmul(
                                reg, ident_sb[:], eb[:, s, :], start=True, stop=False
                            )
                            kw = (
                                dict(tile_position=(lo, 0))
                                if use_tile_position
                                else {}
                            )
                            nc.tensor.matmul(
                                reg,
                                kT[ct][lo : lo + DH, jt * P : (jt + 1) * P],
                                qT[ct][lo : lo + DH, :],
                                start=False,
                                stop=True,
                                **kw,
                            )
                        at = attn_pool.tile([P, 2 * NQ], F16, tag="attn", name="attn")
                        nc.scalar.activation(out=at[:], in_=dps[:], func=AF.Exp)
                        tiles.append(at)
                    return tiles

                def emit_av_pair(hp, tiles_pair):
                    """AV matmuls + normalization + gating for heads 2hp,2hp+1."""
                    ct = hp
                    for s in range(2):
                        h = 2 * hp + s
                        lo = s * DH
                        av = ps_av.tile([P, NQ], F32, tag="av", name="av")
                        for jt in range(JT):
                            nc.tensor.matmul(
                                av[:],
                                v_aug[jt][:, h * P : (h + 1) * P],
                                tiles_pair[jt][:, s * NQ : (s + 1) * NQ],
                                start=(jt == 0),
                                stop=(jt == JT - 1),
                            )
                        rec = rec_pool.tile([DH, NQ], F32, tag="rec", name="rec")
                        nc.vector.reciprocal(out=rec[:], in_=av[DH:P, :])
                        gh = gatedT[ct][lo : lo + DH, :]
                        nc.vector.tensor_tensor(gh, av[0:DH, :], rec[:], ALU.mult)
                        nc.vector.tensor_tensor(
                            gh, gh, sigT[ct][lo : lo + DH, :], ALU.mult
                        )

                prev = None
                for hp in range(HEADS // 2):
                    tiles_pair = emit_dots_pair(hp)
                    if prev is not None:
                        emit_av_pair(hp - 1, prev)
                    prev = tiles_pair
                emit_av_pair(HEADS // 2 - 1, prev)

            # ---- output projection (bo added on host) -------------------
            with tc.tile_pool(name="ps_y", bufs=2, space="PSUM") as ps_y:
                for it in range(NQ // P):
                    ps = ps_y.tile([P, 512], F32, tag="y", name="y")
                    for ct in range(DT):
                        nc.tensor.matmul(
                            ps[:],
                            gatedT[ct][:, it * P : (it + 1) * P],
                            woT[ct][:],
                            start=(ct == 0),
                            stop=(ct == DT - 1),
                        )
                    ysb = yout.tile([P, 512], F32, tag="ysb", name="ysb")
                    nc.any.tensor_copy(out=ysb[:], in_=ps[:])
                    nc.sync.dma_start(y_d[it * P : (it + 1) * P, :], ysb[:])

    nc.compile()
    return nc


_CACHE = {}


def get_nc():
    if "nc" not in _CACHE:
        _CACHE["nc"] = build_nc()
    return _CACHE["nc"]


def make_in_maps(x, attn_bias, wq, wkv, wo, wg, bg):
    """Host-side sharding: per-core input dicts (weights shared by reference)."""
    x = np.asarray(x, np.float32)
    attn_bias = np.asarray(attn_bias, np.float32)
    scale = DH ** -0.5
    wqT = np.ascontiguousarray(np.asarray(wq, np.float32).T * scale)
    wkvT = np.asarray(wkv, np.float32).T
    wkT = np.ascontiguousarray(wkvT[:, :DIM])
    wvT = np.ascontiguousarray(wkvT[:, DIM:])
    wgT = np.ascontiguousarray(np.asarray(wg, np.float32).T)
    woT = np.ascontiguousarray(np.asarray(wo, np.float32).T)
    bg = np.asarray(bg, np.float32)
    ones_v = np.ones((P, DH), np.float16)

    ab = attn_bias[0]  # [H, N(i), N(j)]
    # bT[r0][h, j, i] = bias[h, i, j] with j permuted "query half first"
    bT = {}
    for r0 in (0, NQ):
        perm = np.r_[r0 : r0 + NQ, (NQ - r0) : (NQ - r0) + NQ]
        t = ab[:, r0 : r0 + NQ, :].transpose(0, 2, 1)[:, perm, :]
        bT[r0] = np.ascontiguousarray(t, dtype=np.float16)
    ident = np.eye(P, dtype=np.float16)

    in_maps = []
    for c in range(N_CORES):
        b, r0 = c // 2, (c % 2) * NQ
        perm = np.r_[r0 : r0 + NQ, (NQ - r0) : (NQ - r0) + NQ]
        xbT_c = np.ascontiguousarray(x[b][perm].T)
        in_maps.append(
            {
                "xbT": xbT_c,
                "bT": bT[r0],
                "ident": ident,
                "wqT": wqT,
                "wkT": wkT,
                "wvT": wvT,
                "wgT": wgT,
                "woT": woT,
                "bg": bg,
                "ones_v": ones_v,
            }
        )
    return in_maps


def kernel(x, mask, attn_bias, wq, wkv, wo, bo, wg, bg, **_):
    # mask is all-ones per the problem spec; ignored.
    nc = get_nc()
    in_maps = make_in_maps(x, attn_bias, wq, wkv, wo, wg, bg)
    res = run_bass_kernel_spmd(nc, in_maps, list(range(N_CORES))).results
    y = np.empty((B, N, DIM), np.float32)
    for c in range(N_CORES):
        b, r0 = c // 2, (c % 2) * NQ
        y[b, r0 : r0 + NQ] = res[c]["y"]
    y += np.asarray(bo, np.float32)
    return y


# revision 16
# speedup vs baseline: 1.4030x; 1.4030x over previous
"""Gated multi-head self-attention on 8 Trainium2 NeuronCores.

Reference computation (per batch b of 4, N=1024 tokens, 8 heads x 64):
    q  = (x @ wq.T) * 64**-0.5            # scale folded into wqT on host
    k,v = split(x @ wkv.T)
    dots = q k^T + bias;  attn = softmax(dots)
    out  = (attn @ v) * sigmoid(x @ wg.T + bg)
    y    = out @ wo.T + bo                # bo added on host after gather

Sharding: token-sharded, zero collectives. Core c handles batch b=c//2 and
query-token half c%2 (512 query rows). Each core computes K/V for its whole
batch (the KV projection is duplicated across the 2 cores of a batch; this
is far cheaper than any collective at these sizes).

Layout trick: everything on-device lives channel-major ("transposed") --
qT/kT from wT.T @ xT, the attention output directly in [channel, token]
form -- so no on-device transposes are needed anywhere.  Softmax is done
unnormalized (exp with no max-subtraction: logits are O(5), safe in fp32)
as exp(dots) * exp(bias) where exp(bias^T) is precomputed on host in fp16.
Denominators come for free from 64 ones-columns appended to V inside the
AV matmul (PSUM rows 64:128 = 64 copies of the softmax denominator).
All matmuls run in float32r (full-rate at free dim >= 256).
"""

import sys

if "/opt/trn_rl_repo" not in sys.path:
    sys.path.insert(0, "/opt/trn_rl_repo")

import numpy as np

import concourse.bass as bass  # noqa: F401  (AP helpers)
import concourse.mybir as mybir
import concourse.tile as tile
from concourse import bacc
from concourse.bass_utils import run_bass_kernel_spmd

F32 = mybir.dt.float32
F32R = mybir.dt.float32r
F16 = mybir.dt.float16
AF = mybir.ActivationFunctionType
ALU = mybir.AluOpType

P = 128
HEADS = 8
DH = 64
DIM = 512
N = 1024  # tokens per batch (kv length)
NQ = 512  # query tokens per core
B = 4
N_CORES = 8


def build_nc(use_tile_position=True):
    nc = bacc.Bacc(None, target_bir_lowering=False, debug=False)

    # Per-core inputs. Token order inside xbT/ebT is "query half first".
    xbT_d = nc.dram_tensor("xbT", [DIM, N], F16, kind="ExternalInput")
    bT_d = nc.dram_tensor("bT", [HEADS, N, NQ], F16, kind="ExternalInput")
    wqT_d = nc.dram_tensor("wqT", [DIM, DIM], F16, kind="ExternalInput")
    wkT_d = nc.dram_tensor("wkT", [DIM, DIM], F16, kind="ExternalInput")
    wvT_d = nc.dram_tensor("wvT", [DIM, DIM], F16, kind="ExternalInput")
    wgT_d = nc.dram_tensor("wgT", [DIM, DIM], F16, kind="ExternalInput")
    woT_d = nc.dram_tensor("woT", [DIM, DIM], F16, kind="ExternalInput")
    bg_d = nc.dram_tensor("bg", [DIM], F32, kind="ExternalInput")
    ones_d = nc.dram_tensor("ones_v", [P, DH], F16, kind="ExternalInput")
    y_d = nc.dram_tensor("y", [NQ, DIM], F32, kind="ExternalOutput")

    DT = DIM // P  # 4 channel tiles of 128
    JT = N // P  # 8 kv-token tiles of 128

    with tile.TileContext(nc) as tc:
        with (
            tc.tile_pool(name="const", bufs=1) as const,
            tc.tile_pool(name="work", bufs=1) as work,
            tc.tile_pool(name="attn", bufs=16) as attn_pool,
            tc.tile_pool(name="rec", bufs=4) as rec_pool,
            tc.tile_pool(name="ebuf", bufs=6) as ebuf,
            tc.tile_pool(name="yout", bufs=2) as yout,
        ):
            # ---- constants ----------------------------------------------
            xbT = [const.tile([P, N], F16, tag=f"xbT{t}", name=f"xbT{t}") for t in range(DT)]
            for t in range(DT):
                nc.sync.dma_start(xbT[t][:], xbT_d[t * P : (t + 1) * P, :])

            def load_w(name, dram, eng):
                ts_ = [const.tile([P, DIM], F16, tag=f"{name}{t}", name=f"{name}{t}") for t in range(DT)]
                for t in range(DT):
                    eng.dma_start(ts_[t][:], dram[t * P : (t + 1) * P, :])
                return ts_

            wkT = load_w("wkT", wkT_d, nc.scalar)
            wvT = load_w("wvT", wvT_d, nc.gpsimd)
            wqT = load_w("wqT", wqT_d, nc.scalar)
            wgT = load_w("wgT", wgT_d, nc.gpsimd)
            woT = load_w("woT", woT_d, nc.scalar)

            bg_sb = const.tile([P, DT], F32, tag="bg", name="bg")
            nc.sync.dma_start(bg_sb[:], bg_d.rearrange("(o p) -> p o", p=P))

            # persistent activations
            kT = [work.tile([P, N], F16, tag=f"kT{t}", name=f"kT{t}") for t in range(DT)]
            v_aug = [work.tile([P, HEADS * P], F16, tag=f"vaug{j}", name=f"vaug{j}") for j in range(JT)]
            qT = [work.tile([P, NQ], F16, tag=f"qT{t}", name=f"qT{t}") for t in range(DT)]
            sigT = [work.tile([P, NQ], F16, tag=f"sig{t}", name=f"sig{t}") for t in range(DT)]
            gatedT = [work.tile([P, NQ], F16, tag=f"gated{t}", name=f"gated{t}") for t in range(DT)]

            # ---- PE warmup: get HAM to K=8/8 during the initial DMAs ----
            warm_sb = const.tile([P, 512], F16, tag="warm", name="warm")
            nc.sync.dma_start(
                warm_sb[:], ones_d[:, None, :].to_broadcast((P, HEADS, DH))
            )

            # ---- projections --------------------------------------------
            with (
                tc.tile_pool(name="ps_warm", bufs=1, space="PSUM") as ps_warm,
                tc.tile_pool(name="ps_proj", bufs=4, space="PSUM") as ps_proj,
            ):
                warm_ps = ps_warm.tile([P, 512], F32, tag="warm", name="warm_ps")
                for _ in range(16):
                    nc.tensor.matmul(
                        warm_ps[:], warm_sb[:, 0:P], warm_sb[:], start=True, stop=True
                    )
                # consumer so the warmup chain is not dead code
                nc.any.tensor_copy(out=warm_sb[0:1, 0:1], in_=warm_ps[0:1, 0:1])

                # kT[ct][:, j] = sum_d wkT[d, ct].T @ xbT[d, j]
                for ct in range(DT):
                    for jc in range(2):
                        ps = ps_proj.tile([P, 512], F32, tag="proj", name="proj")
                        for kt in range(DT):
                            nc.tensor.matmul(
                                ps[:],
                                wkT[kt][:, ct * P : (ct + 1) * P],
                                xbT[kt][:, jc * 512 : (jc + 1) * 512],
                                start=(kt == 0),
                                stop=(kt == DT - 1),
                            )
                        nc.any.tensor_copy(
                            out=kT[ct][:, jc * 512 : (jc + 1) * 512], in_=ps[:]
                        )

                # v_aug[j]: per head h cols h*128..h*128+64 = V, +64..+128 = ones
                for jt in range(JT):
                    nc.sync.dma_start(
                        v_aug[jt].rearrange("p (h c) -> p h c", c=P)[:, :, DH:P],
                        ones_d[:, None, :].to_broadcast((P, HEADS, DH)),
                    )
                for jt in range(JT):
                    ps = ps_proj.tile([P, 512], F32, tag="proj", name="proj")
                    for kt in range(DT):
                        nc.tensor.matmul(
                            ps[:],
                            xbT[kt][:, jt * P : (jt + 1) * P],
                            wvT[kt][:],
                            start=(kt == 0),
                            stop=(kt == DT - 1),
                        )
                    nc.any.tensor_copy(
                        out=v_aug[jt].rearrange("p (h c) -> p h c", c=P)[:, :, 0:DH],
                        in_=ps[:].rearrange("p (h c) -> p h c", c=DH),
                    )

                # qT (wq already scaled by 1/8 on host)
                for ct in range(DT):
                    ps = ps_proj.tile([P, 512], F32, tag="proj", name="proj")
                    for kt in range(DT):
                        nc.tensor.matmul(
                            ps[:],
                            wqT[kt][:, ct * P : (ct + 1) * P],
                            xbT[kt][:, 0:NQ],
                            start=(kt == 0),
                            stop=(kt == DT - 1),
                        )
                    nc.any.tensor_copy(out=qT[ct][:], in_=ps[:])

                # gates -> sigmoid(g + bg) directly from PSUM
                for ct in range(DT):
                    ps = ps_proj.tile([P, 512], F32, tag="proj", name="proj")
                    for kt in range(DT):
                        nc.tensor.matmul(
                            ps[:],
                            wgT[kt][:, ct * P : (ct + 1) * P],
                            xbT[kt][:, 0:NQ],
                            start=(kt == 0),
                            stop=(kt == DT - 1),
                        )
                    nc.scalar.activation(
                        out=sigT[ct][:],
                        in_=ps[:],
                        func=AF.Sigmoid,
                        bias=bg_sb[:, ct : ct + 1],
                    )

            # ---- attention (head pairs) ---------------------------------
            with (
                tc.tile_pool(name="ps_dots", bufs=2, space="PSUM") as ps_dots,
                tc.tile_pool(name="ps_av", bufs=4, space="PSUM") as ps_av,
            ):
                def emit_dots_pair(hp):
                    """QK^T + exp(dots)*exp(bias) for heads 2hp, 2hp+1.

                    Returns 8 fp16 attn tiles [P(j), 2*NQ] (even head cols
                    0:NQ, odd head NQ:2NQ), unnormalized exp(dots+bias)."""
                    ct = hp  # channel tile holding this head pair
                    tiles = []
                    for jt in range(JT):
                        eb = ebuf.tile([P, 2, NQ], F16, tag="eb", name="eb")
                        nc.sync.dma_start(
                            eb[:],
                            bT_d[2 * hp : 2 * hp + 2, jt * P : (jt + 1) * P, :]
                            .rearrange("h p i -> p h i"),
                        )
                        dps = ps_dots.tile([P, 2 * NQ], F32, tag="dots", name="dots")
                        for s in range(2):
                            lo = s * DH
                            reg = dps[:, s * NQ : (s + 1) * NQ]
                            kw = (
                                dict(tile_position=(lo, 0))
                                if use_tile_position
                                else {}
                            )
                            nc.tensor.matmul(
                                reg,
                                kT[ct][lo : lo + DH, jt * P : (jt + 1) * P],
                                qT[ct][lo : lo + DH, :],
                                start=True,
                                stop=True,
                                **kw,
                            )
                        at = attn_pool.tile([P, 2 * NQ], F16, tag="attn", name="attn")
                        nc.scalar.activation(out=at[:], in_=dps[:], func=AF.Exp)
                        nc.vector.tensor_tensor(
                            at[:], at[:], eb[:, :, :].rearrange("p h i -> p (h i)"), ALU.mult
                        )
                        tiles.append(at)
                    return tiles

                def emit_av_pair(hp, tiles_pair):
                    """AV matmuls + normalization + gating for heads 2hp,2hp+1."""
                    ct = hp
                    for s in range(2):
                        h = 2 * hp + s
                        lo = s * DH
                        av = ps_av.tile([P, NQ], F32, tag="av", name="av")
                        for jt in range(JT):
                            nc.tensor.matmul(
                                av[:],
                                v_aug[jt][:, h * P : (h + 1) * P],
                                tiles_pair[jt][:, s * NQ : (s + 1) * NQ],
                                start=(jt == 0),
                                stop=(jt == JT - 1),
                            )
                        rec = rec_pool.tile([DH, NQ], F32, tag="rec", name="rec")
                        nc.vector.reciprocal(out=rec[:], in_=av[DH:P, :])
                        gh = gatedT[ct][lo : lo + DH, :]
                        nc.vector.tensor_tensor(gh, av[0:DH, :], rec[:], ALU.mult)
                        nc.vector.tensor_tensor(
                            gh, gh, sigT[ct][lo : lo + DH, :], ALU.mult
                        )

                prev = None
                for hp in range(HEADS // 2):
                    tiles_pair = emit_dots_pair(hp)
                    if prev is not None:
                        emit_av_pair(hp - 1, prev)
                    prev = tiles_pair
                emit_av_pair(HEADS // 2 - 1, prev)

            # ---- output projection (bo added on host) -------------------
            with tc.tile_pool(name="ps_y", bufs=2, space="PSUM") as ps_y:
                for it in range(NQ // P):
                    ps = ps_y.tile([P, 512], F32, tag="y", name="y")
                    for ct in range(DT):
                        nc.tensor.matmul(
                            ps[:],
                            gatedT[ct][:, it * P : (it + 1) * P],
                            woT[ct][:],
                            start=(ct == 0),
                            stop=(ct == DT - 1),
                        )
                    ysb = yout.tile([P, 512], F32, tag="ysb", name="ysb")
                    nc.any.tensor_copy(out=ysb[:], in_=ps[:])
                    nc.sync.dma_start(y_d[it * P : (it + 1) * P, :], ysb[:])

    nc.compile()
    return nc


_CACHE = {}


def get_nc():
    if "nc" not in _CACHE:
        _CACHE["nc"] = build_nc()
    return _CACHE["nc"]


def make_in_maps(x, attn_bias, wq, wkv, wo, wg, bg):
    """Host-side sharding: per-core input dicts (weights shared by reference)."""
    x = np.asarray(x, np.float32)
    attn_bias = np.asarray(attn_bias, np.float32)
    scale = DH ** -0.5
    wqT = np.ascontiguousarray((np.asarray(wq, np.float32).T * scale), np.float16)
    wkvT = np.asarray(wkv, np.float32).T
    wkT = np.ascontiguousarray(wkvT[:, :DIM], np.float16)
    wvT = np.ascontiguousarray(wkvT[:, DIM:], np.float16)
    wgT = np.ascontiguousarray(np.asarray(wg, np.float32).T, np.float16)
    woT = np.ascontiguousarray(np.asarray(wo, np.float32).T, np.float16)
    bg = np.asarray(bg, np.float32)
    ones_v = np.ones((P, DH), np.float16)

    ab = np.exp(attn_bias[0])  # [H, N(i), N(j)]
    # bT[r0][h, j, i] = exp(bias)[h, i, j] with j permuted "query half first"
    bT = {}
    for r0 in (0, NQ):
        perm = np.r_[r0 : r0 + NQ, (NQ - r0) : (NQ - r0) + NQ]
        t = ab[:, r0 : r0 + NQ, :].transpose(0, 2, 1)[:, perm, :]
        bT[r0] = np.ascontiguousarray(t, dtype=np.float16)

    in_maps = []
    for c in range(N_CORES):
        b, r0 = c // 2, (c % 2) * NQ
        perm = np.r_[r0 : r0 + NQ, (NQ - r0) : (NQ - r0) + NQ]
        xbT_c = np.ascontiguousarray(x[b][perm].T, np.float16)
        in_maps.append(
            {
                "xbT": xbT_c,
                "bT": bT[r0],
                "wqT": wqT,
                "wkT": wkT,
                "wvT": wvT,
                "wgT": wgT,
                "woT": woT,
                "bg": bg,
                "ones_v": ones_v,
            }
        )
    return in_maps


def kernel(x, mask, attn_bias, wq, wkv, wo, bo, wg, bg, **_):
    # mask is all-ones per the problem spec; ignored.
    nc = get_nc()
    in_maps = make_in_maps(x, attn_bias, wq, wkv, wo, wg, bg)
    res = run_bass_kernel_spmd(nc, in_maps, list(range(N_CORES))).results
    y = np.empty((B, N, DIM), np.float32)
    for c in range(N_CORES):
        b, r0 = c // 2, (c % 2) * NQ
        y[b, r0 : r0 + NQ] = res[c]["y"]
    y += np.asarray(bo, np.float32)
    return y
